# revision 1
# baseline (speedup 1.0000x reference)
"""BERT_BiLSTM_CRF loss (CRF NLL) Trainium2 kernel.

Self-contained: kernel(**inputs) takes FULL inputs, shards batch across 8
NeuronCores, runs a Bass/Tile kernel per core, returns the scalar mean loss.

Algorithm (validated vs reference in fp32 numpy):
  Forward scores via exp-space renormalized recurrence over the 7 active
  states (states 7=START / 8=STOP are exactly inert in fp32 because the
  -10000 transition rows/cols underflow exp to exactly 0):
      S_0[j]   = exp(trans[j,7]) * exp(feat[0,j] - G)
      S_t[j]   = sum_i S_{t-1}[i] * exp(trans[j,i]) * exp(feat[t,j] - G)
      renorm every RN steps: S /= max(S), accumulate log(max) in offh
      alpha_t  = log(S_t) + G*(t+1) + offh_cum
  Per-step work on the Pool engine: one broadcast-multiply [128,49] and one
  segmented reduce [128,49]->[128,7], reading per-step matrices
  M_t[j,i] = exp(trans[j,i]) * exp(feat[t,j]-G) that are precomputed in bulk
  on DVE. History S_t is written to DRAM; final per-sequence state gathered
  at t = len-1 with an indirect DMA.
  Gold scores via one-hot dot products (TensorTensorReduce on DVE) and
  transition pair-counts N[j,i] (49 TTRs over a state-major bf16 one-hot).
"""

import numpy as np

B, T, K = 1024, 2048, 9
NCORES = 8
BL = B // NCORES          # sequences per core (=128 partitions)
KA = 7                    # active states
START, STOP = 7, 8
G = 2.4                   # expected per-step log growth (numerical centering)
RN = 32                   # renorm interval
NRE = T // RN             # renorm events
CT = 128                  # forward/time chunk
NCH = T // CT

_CACHE = {}
NO_INDIRECT = False
TRACE = False
NCH_USE = None


def _build_bass():
    import concourse.bass as bass
    import concourse.bacc as bacc
    import concourse.tile as tile
    import concourse.mybir as mybir

    f32 = mybir.dt.float32
    bf16 = mybir.dt.bfloat16
    i32 = mybir.dt.int32
    AX = mybir.AxisListType
    OP = mybir.AluOpType
    AF = mybir.ActivationFunctionType

    nc = bacc.Bacc()

    feats = nc.dram_tensor("feats", [BL, T, K], f32, kind="ExternalInput")
    tagf = nc.dram_tensor("tagf", [BL, T], f32, kind="ExternalInput")
    lenf = nc.dram_tensor("lenf", [BL, 1], f32, kind="ExternalInput")
    leni = nc.dram_tensor("leni", [BL, 1], i32, kind="ExternalInput")
    trans = nc.dram_tensor("trans", [K, K], f32, kind="ExternalInput")
    outv = nc.dram_tensor("outv", [BL, 1], f32, kind="ExternalOutput")

    ahist = nc.dram_tensor("ahist", [BL * T, KA], f32)
    offh_d = nc.dram_tensor("offh_d", [BL * (NRE + 1), 1], f32)

    # host-side constants embedded in the NEFF
    iota_t_np = np.arange(T, dtype=np.float32).reshape(1, T)
    iotaPTm1_np = (np.arange(BL, dtype=np.int64) * T - 1).astype(np.int32).reshape(BL, 1)
    iotaP33_np = (np.arange(BL, dtype=np.int64) * (NRE + 1)).astype(np.int32).reshape(BL, 1)
    c_iota_t = nc.inline_tensor(iota_t_np, "c_iota_t")
    c_iotaPTm1 = nc.inline_tensor(iotaPTm1_np, "c_iotaPTm1")
    c_iotaP33 = nc.inline_tensor(iotaP33_np, "c_iotaP33")

    with tile.TileContext(nc) as tc:
        import contextlib
        ctx = contextlib.ExitStack()
        with ctx:
            singles = ctx.enter_context(tc.tile_pool(name="singles", bufs=1))
            fpool = ctx.enter_context(tc.tile_pool(name="fpool", bufs=2))
            mpool = ctx.enter_context(tc.tile_pool(name="mpool", bufs=2))
            hpool = ctx.enter_context(tc.tile_pool(name="hpool", bufs=3))
            bigp = ctx.enter_context(tc.tile_pool(name="bigp", bufs=4))
            smallp = ctx.enter_context(tc.tile_pool(name="smallp", bufs=4))

            # ---- constants in SBUF ----
            transb = singles.tile([BL, K * K], f32)     # raw trans, replicated
            nc.gpsimd.dma_start(transb[:], bass.AP(trans, 0, [[0, BL], [1, K * K]]))
            iota_t = singles.tile([BL, T], f32)
            nc.gpsimd.dma_start(iota_t[:], bass.AP(c_iota_t, 0, [[0, BL], [1, T]]))
            iotaPTm1 = singles.tile([BL, 1], i32)
            nc.gpsimd.dma_start(iotaPTm1[:], c_iotaPTm1[:, :])
            iotaP33 = singles.tile([BL, 1], i32)
            nc.gpsimd.dma_start(iotaP33[:], c_iotaP33[:, :])
            lenf_sb = singles.tile([BL, 1], f32)
            nc.gpsimd.dma_start(lenf_sb[:], lenf[:, :])
            leni_sb = singles.tile([BL, 1], i32)
            nc.gpsimd.dma_start(leni_sb[:], leni[:, :])

            trv = transb[:].rearrange("p (j i) -> p j i", i=K)
            tr49 = trv[:, 0:KA, 0:KA]                    # [p,7,7] raw
            tr7col = trv[:, 0:KA, START:START + 1]       # [p,7,1] trans[j,7]
            tr8row = trv[:, STOP:STOP + 1, 0:KA]         # [p,1,7] trans[8,j]

            # exp() constants on ACT
            Eb = singles.tile([BL, KA * KA], f32)        # exp(trans[j,i])
            nc.scalar.activation(Eb[:], tr49, AF.Exp)
            E7E = singles.tile([BL, KA], f32)            # exp(trans[j,7])
            nc.scalar.activation(E7E[:], tr7col, AF.Exp)
            E8E = singles.tile([BL, KA], f32)            # exp(trans[8,j])
            nc.scalar.activation(E8E[:], tr8row, AF.Exp)
            Ebv = Eb[:].rearrange("p (j i) -> p j i", i=KA)

            negG = singles.tile([BL, 1], f32)
            nc.vector.memset(negG[:], -G)

            # one-hot tag stores, state-major [p, j, t], bf16, resident
            ohF = singles.tile([BL, KA, T], bf16)
            ohmF = singles.tile([BL, KA, T], bf16)
            offsb = singles.tile([BL, NRE + 1], f32)
            nc.vector.memset(offsb[:, 0:1], 0.0)

            # gold accumulators (ping-pong chained TTR)
            fpcols = singles.tile([BL, NCH], f32)
            featp = singles.tile([BL, 1], f32)

            junkC = bigp.tile([BL, CT, KA], bf16, tag="junkC")  # TTR main out (reused)
            junkT0 = singles.tile([BL, T - 1], bf16)
            junkT1 = singles.tile([BL, T - 1], bf16)
            Ntile = singles.tile([BL, KA * KA], f32)

            hist_tiles = []
            prev_slot = None

            nch_use = NCH if NCH_USE is None else NCH_USE
            for c in range(nch_use):
                t0 = c * CT
                featsc = fpool.tile([BL, CT, K], f32, tag="featsc")
                nc.sync.dma_start(featsc[:], feats[:, t0:t0 + CT, :])
                tagfc = fpool.tile([BL, CT], f32, tag="tagfc")
                nc.sync.dma_start(tagfc[:], tagf[:, t0:t0 + CT])

                # ef = exp(feat - G) on ACT  [p, CT*7]
                efc = fpool.tile([BL, CT, KA], f32, tag="efc")
                nc.scalar.activation(efc[:], featsc[:, :, 0:KA], AF.Exp,
                                     bias=negG[:, 0:1])

                # M[t,j,i] = Eb[j,i] * ef[t,j]  on DVE  [p, CT,7,7]
                Mc = mpool.tile([BL, CT, KA, KA], f32, tag="Mc")
                nc.gpsimd.tensor_tensor(
                    Mc[:],
                    efc[:].unsqueeze(3).broadcast_to([BL, CT, KA, KA]),
                    Ebv.unsqueeze(1).broadcast_to([BL, CT, KA, KA]),
                    op=mybir.AluOpType.mult,
                )

                # one-hot (Pool; fp32 -> bf16 conversion allowed on Pool)
                ohslice = ohF[:, :, t0:t0 + CT].rearrange("p j t -> p t j")
                nc.vector.tensor_tensor(
                    ohslice,
                    tagfc[:].unsqueeze(2).broadcast_to([BL, CT, KA]),
                    iota_t[:, 0:KA].unsqueeze(1).broadcast_to([BL, CT, KA]),
                    op=OP.is_equal,
                )
                # mask (Pool): (t < len) as bf16
                maskc = fpool.tile([BL, CT], bf16, tag="maskc")
                nc.vector.tensor_tensor(
                    maskc[:], iota_t[:, t0:t0 + CT],
                    lenf_sb[:].broadcast_to([BL, CT]), op=OP.is_lt,
                )
                # ohm = oh * mask (Pool, bf16, state-major contiguous)
                nc.gpsimd.tensor_tensor(
                    ohmF[:, :, t0:t0 + CT],
                    ohF[:, :, t0:t0 + CT],
                    maskc[:].unsqueeze(1).broadcast_to([BL, KA, CT]),
                    op=OP.mult,
                )

                # gold feat part: TTR( bf16(feats), ohm ) accumulated across chunks
                featsb = fpool.tile([BL, CT, KA], bf16, tag="featsb")
                nc.scalar.activation(featsb[:], featsc[:, :, 0:KA], AF.Copy)
                nc.gpsimd.tensor_tensor(
                    junkC[:],
                    featsb[:],
                    ohmF[:, :, t0:t0 + CT].rearrange("p j t -> p t j"),
                    op=OP.mult,
                )
                nc.vector.tensor_reduce(
                    out=fpcols[:, c:c + 1], in_=junkC[:].rearrange("p t j -> p (t j)"),
                    axis=AX.X, op=OP.add)

                # ---- forward recurrence over this chunk (Pool) ----
                histc = hpool.tile([BL, CT, KA], f32, tag="hist")
                hist_tiles.append(histc)
                for l in range(CT):
                    t = t0 + l
                    slot = histc[:, l, :]
                    if t == 0:
                        nc.vector.tensor_tensor(
                            slot, E7E[:], efc[:, 0, :], op=OP.mult)
                    else:
                        sprev = prev_slot.unsqueeze(1).broadcast_to([BL, KA, KA])
                        big = bigp.tile([BL, KA, KA], f32, tag="big")
                        nc.vector.tensor_tensor(
                            big[:], sprev, Mc[:, l, :, :], op=OP.mult)
                        nc.vector.tensor_reduce(
                            out=slot, in_=big[:], axis=AX.X, op=OP.add)
                    if (t + 1) % RN == 0:
                        kre = (t + 1) // RN
                        mx = smallp.tile([BL, 1], f32, tag="mx")
                        nc.vector.tensor_reduce(
                            out=mx[:], in_=slot, axis=AX.X, op=OP.max)
                        rc = smallp.tile([BL, 1], f32, tag="rc")
                        nc.vector.reciprocal(rc[:], mx[:])
                        nc.vector.tensor_tensor(
                            slot, slot, rc[:].broadcast_to([BL, KA]),
                            op=OP.mult)
                        lnm = smallp.tile([BL, 1], f32, tag="lnm")
                        nc.scalar.activation(lnm[:], mx[:], AF.Ln)
                        nc.gpsimd.tensor_tensor(
                            offsb[:, kre:kre + 1], lnm[:],
                            offsb[:, kre - 1:kre], op=OP.add)
                    prev_slot = histc[:, l, :]
                # flush chunk history to DRAM
                nc.sync.dma_start(
                    bass.AP(ahist, t0 * KA, [[T * KA, BL], [1, CT * KA]]),
                    histc[:].rearrange("p t j -> p (t j)"),
                )

            # ---- gold transition pair counts: N[j,i] = sum_t ohm[t,j]*oh[t-1,i]
            for j in range(KA):
                for i in range(KA):
                    jk = (junkT0, junkT1)[(j * KA + i) % 2]
                    nc.gpsimd.tensor_tensor(
                        jk[:], ohmF[:, j, 1:T], ohF[:, i, 0:T - 1], op=OP.mult)
                    nc.vector.tensor_reduce(
                        out=Ntile[:, j * KA + i:j * KA + i + 1], in_=jk[:],
                        axis=AX.X, op=OP.add)
            transdot = smallp.tile([BL, 1], f32, tag="transdot")
            junk49 = smallp.tile([BL, KA * KA], f32, tag="junk49")
            tr49c = smallp.tile([BL, KA * KA], f32, tag="tr49c")
            nc.gpsimd.tensor_copy(tr49c[:], tr49)
            nc.gpsimd.tensor_tensor(junk49[:], Ntile[:], tr49c[:], op=OP.mult)
            nc.vector.tensor_reduce(
                out=transdot[:, 0:1], in_=junk49[:], axis=AX.X, op=OP.add)
            # t0 term: trans[tag_0, START]
            oh0f = smallp.tile([BL, KA], f32, tag="oh0f")
            nc.gpsimd.tensor_copy(oh0f[:], ohF[:, :, 0:1].rearrange("p j t -> p (j t)"))
            t0p = smallp.tile([BL, 1], f32, tag="t0p")
            junk7 = smallp.tile([BL, KA], f32, tag="junk7")
            nc.gpsimd.tensor_tensor(
                junk7[:], oh0f[:], tr7col.rearrange("p j o -> p (j o)"), op=OP.mult)
            nc.vector.tensor_reduce(
                out=t0p[:, 0:1], in_=junk7[:], axis=AX.X, op=OP.add)

            # ---- final gathers ----
            idxA = smallp.tile([BL, 1], i32, tag="idxA")
            nc.vector.tensor_tensor(idxA[:], iotaPTm1[:], leni_sb[:], op=OP.add)
            Sg = smallp.tile([BL, KA], f32, tag="Sg")
            if NO_INDIRECT:
                nc.sync.dma_start(Sg[:], bass.AP(ahist, 0, [[T * KA, BL], [1, KA]]))
            else:
                nc.gpsimd.indirect_dma_start(
                    out=Sg[:], out_offset=None,
                    in_=bass.AP(ahist, 0, [[KA, BL * T], [1, KA]]),
                    in_offset=bass.IndirectOffsetOnAxis(ap=idxA[:, 0:1], axis=0),
                )
            # last tag gather + term trans[STOP, tag_last]
            tglf = smallp.tile([BL, 1], f32, tag="tglf")
            if NO_INDIRECT:
                nc.sync.dma_start(tglf[:], bass.AP(tagf, 0, [[T, BL], [1, 1]]))
            else:
                nc.gpsimd.indirect_dma_start(
                    out=tglf[:], out_offset=None,
                    in_=bass.AP(tagf, 0, [[1, BL * T], [1, 1]]),
                    in_offset=bass.IndirectOffsetOnAxis(ap=idxA[:, 0:1], axis=0),
                )
            ohlast = smallp.tile([BL, KA], f32, tag="ohlast")
            nc.vector.tensor_tensor(
                ohlast[:], tglf[:].broadcast_to([BL, KA]), iota_t[:, 0:KA],
                op=OP.is_equal)
            lastp = smallp.tile([BL, 1], f32, tag="lastp")
            junk7b = smallp.tile([BL, KA], f32, tag="junk7b")
            nc.gpsimd.tensor_tensor(
                junk7b[:], ohlast[:], tr8row.rearrange("p o j -> p (o j)"), op=OP.mult)
            nc.vector.tensor_reduce(
                out=lastp[:, 0:1], in_=junk7b[:], axis=AX.X, op=OP.add)

            # offh: flush + gather at k = len >> 6
            nc.sync.dma_start(
                bass.AP(offh_d, 0, [[NRE + 1, BL], [1, NRE + 1]]), offsb[:])
            c6 = smallp.tile([BL, 1], i32, tag="c6")
            nc.vector.memset(c6[:], 5)
            ksh = smallp.tile([BL, 1], i32, tag="ksh")
            nc.vector.tensor_tensor(ksh[:], leni_sb[:], c6[:], op=OP.logical_shift_right)
            idxB = smallp.tile([BL, 1], i32, tag="idxB")
            nc.vector.tensor_tensor(idxB[:], iotaP33[:], ksh[:], op=OP.add)
            offg = smallp.tile([BL, 1], f32, tag="offg")
            if NO_INDIRECT:
                nc.sync.dma_start(offg[:], bass.AP(offh_d, 0, [[NRE + 1, BL], [1, 1]]))
            else:
                nc.gpsimd.indirect_dma_start(
                    out=offg[:], out_offset=None,
                    in_=bass.AP(offh_d, 0, [[1, BL * (NRE + 1)], [1, 1]]),
                    in_offset=bass.IndirectOffsetOnAxis(ap=idxB[:, 0:1], axis=0),
                )

            # fwd = ln(sum_j Sg*E8) + offg + G*len
            dotv = smallp.tile([BL, 1], f32, tag="dotv")
            junk7c = smallp.tile([BL, KA], f32, tag="junk7c")
            nc.gpsimd.tensor_tensor(junk7c[:], Sg[:], E8E[:], op=OP.mult)
            nc.vector.tensor_reduce(
                out=dotv[:, 0:1], in_=junk7c[:], axis=AX.X, op=OP.add)
            lnv = smallp.tile([BL, 1], f32, tag="lnv")
            nc.scalar.activation(lnv[:], dotv[:], AF.Ln)
            lnvo = smallp.tile([BL, 1], f32, tag="lnvo")
            nc.vector.tensor_tensor(lnvo[:], lnv[:], offg[:], op=OP.add)
            gconst = smallp.tile([BL, 1], f32, tag="gconst")
            nc.vector.memset(gconst[:], G)
            glen = smallp.tile([BL, 1], f32, tag="glen")
            nc.vector.tensor_tensor(glen[:], lenf_sb[:], gconst[:], op=OP.mult)
            fwdv = smallp.tile([BL, 1], f32, tag="fwdv")
            nc.vector.tensor_tensor(fwdv[:], lnvo[:], glen[:], op=OP.add)

            # gold = featpart + transdot + t0p + lastp
            nc.vector.tensor_reduce(out=featp[:], in_=fpcols[:], axis=AX.X, op=OP.add)
            g1 = smallp.tile([BL, 1], f32, tag="g1")
            nc.vector.tensor_tensor(g1[:], featp[:], transdot[:], op=OP.add)
            g2 = smallp.tile([BL, 1], f32, tag="g2")
            nc.vector.tensor_tensor(g2[:], t0p[:], lastp[:], op=OP.add)
            g3 = smallp.tile([BL, 1], f32, tag="g3")
            nc.vector.tensor_tensor(g3[:], g1[:], g2[:], op=OP.add)
            res = smallp.tile([BL, 1], f32, tag="res")
            nc.vector.tensor_tensor(res[:], fwdv[:], g3[:], op=OP.subtract)
            nc.sync.dma_start(outv[:, :], res[:])

    nc.finalize()
    return nc


def kernel(feats, transitions, tags, lengths):
    feats = np.ascontiguousarray(np.asarray(feats, dtype=np.float32))
    transitions = np.ascontiguousarray(np.asarray(transitions, dtype=np.float32))
    tags_f = np.ascontiguousarray(np.asarray(tags).astype(np.float32))
    len_f = np.ascontiguousarray(np.asarray(lengths).astype(np.float32).reshape(B, 1))
    len_i = np.ascontiguousarray(np.asarray(lengths).astype(np.int32).reshape(B, 1))

    if "nc" not in _CACHE:
        _CACHE["nc"] = _build_bass()
    nc = _CACHE["nc"]

    from concourse.bass_utils import run_bass_kernel_spmd

    in_maps = []
    for c in range(NCORES):
        sl = slice(c * BL, (c + 1) * BL)
        in_maps.append({
            "feats": feats[sl],
            "tagf": tags_f[sl],
            "lenf": len_f[sl],
            "leni": len_i[sl],
            "trans": transitions,
        })
    r = run_bass_kernel_spmd(nc, in_maps, core_ids=list(range(NCORES)),
                             trace=TRACE)
    if TRACE:
        _CACHE["last_result"] = r
    per_seq = np.concatenate([m["outv"].reshape(BL) for m in r.results])
    return np.float32(per_seq.mean(dtype=np.float64))



# revision 7
# speedup vs baseline: 1.6737x; 1.6737x over previous
"""BERT_BiLSTM_CRF loss (CRF NLL) Trainium2 kernel, v2.

Forward-backward meet-in-the-middle CRF forward scores + PE-based gold scores.

Per core (BL=128 seqs on partitions):
  - alpha chain (t=0..1023) and beta chain (q=0..1024) run as ONE dual-batched
    exp-space recurrence on DVE: state [BL,2,7], step matrices Mab [BL,2,7,7]
    built in bulk on Pool from ACT exp(feats-G). beta runs on host-reversed
    per-length zero-padded feats. fwd = ln(alpha_m . beta_q) + offsets for
    long seqs, ln(E8 . alpha_{len-1}) + offsets for short.
  - Renorm by running max every RN dual-steps (in-place on the stored history
    slot, per lane); ln(max) batched on ACT; prefix-sums via DVE scan.
  - History SBUF-resident, one flush to DRAM, per-seq indirect-DMA gathers
    with host-precomputed indices.
  - Gold via per-seq PE matmuls on host-transposed one-hot/feat planes:
    PSUM[j, s, 0:8]  = sum_t ohm[t,j] ohprev8[t,i]  (pair counts, START col)
    PSUM[j, s, 8:15] = sum_t ohm[t,j] featT[t,j']   (diagonal = feat part)
    dotted against [trans[0:7,0:8] | I7]; plus the STOP-row last-tag term.
Output: per-seq (fwd - gold) [BL,1] f32; host takes the mean.
"""

import numpy as np

B, T, K = 1024, 2048, 9
NCORES = 8
BL = B // NCORES
KA = 7
START, STOP = 7, 8
G = 2.4
H = T // 2                 # chain length (dual steps)
RN = 32                    # dual-steps between renorms
NEV = H // RN              # renorm events per lane
CT = 128                   # dual-steps per M-build chunk
NCH = H // CT
TC_ = 128                  # t per transposed gold chunk
NGC = T // TC_             # gold chunks
NPS = 15                   # psum cols per seq (8 pair + 7 feat)

_CACHE = {}
TRACE = False


def _build_bass():
    import contextlib

    import concourse.bass as bass
    import concourse.bacc as bacc
    import concourse.mybir as mybir
    import concourse.tile as tile

    f32 = mybir.dt.float32
    bf16 = mybir.dt.bfloat16
    i32 = mybir.dt.int32
    AX = mybir.AxisListType
    OP = mybir.AluOpType
    AF = mybir.ActivationFunctionType

    nc = bacc.Bacc()

    # ---- DRAM inputs ----
    fwd_f = nc.dram_tensor("fwd_f", [BL, H, KA], bf16, kind="ExternalInput")
    rev_f = nc.dram_tensor("rev_f", [BL, H, KA], bf16, kind="ExternalInput")
    featT = nc.dram_tensor("featT", [TC_, NGC, BL, KA], bf16, kind="ExternalInput")
    tagT = nc.dram_tensor("tagT", [TC_, NGC, BL], bf16, kind="ExternalInput")
    tagpT = nc.dram_tensor("tagpT", [TC_, NGC, BL], bf16, kind="ExternalInput")
    maskT = nc.dram_tensor("maskT", [TC_, NGC, BL], bf16, kind="ExternalInput")
    lenf = nc.dram_tensor("lenf", [BL, 1], f32, kind="ExternalInput")
    hidx = nc.dram_tensor("hidx", [BL, 1], i32, kind="ExternalInput")
    oidx = nc.dram_tensor("oidx", [BL, 1], i32, kind="ExternalInput")
    lidx = nc.dram_tensor("lidx", [BL, 1], i32, kind="ExternalInput")
    islf = nc.dram_tensor("islf", [BL, 1], f32, kind="ExternalInput")
    trans = nc.dram_tensor("trans", [K, K], f32, kind="ExternalInput")
    patR = nc.dram_tensor("patR", [KA, NPS], f32, kind="ExternalInput")
    outv = nc.dram_tensor("outv", [BL, 1], f32, kind="ExternalOutput")

    # DRAM scratch
    dh_d = nc.dram_tensor("dh_d", [BL * (H + 1) * 2, KA], bf16)
    off_d = nc.dram_tensor("off_d", [BL * (NEV + 1), 2], f32)
    gvec_d = nc.dram_tensor("gvec_d", [KA, BL], f32)

    iota7_np = np.arange(KA, dtype=np.float32).reshape(1, KA)
    c_iota7 = nc.inline_tensor(iota7_np, "c_iota7")
    iota8p1_np = np.arange(1, KA + 2, dtype=np.float32).reshape(1, KA + 1)
    c_iota8p1 = nc.inline_tensor(iota8p1_np, "c_iota8p1")

    with tile.TileContext(nc) as tc:
        ctx = contextlib.ExitStack()
        with ctx, nc.allow_low_precision(reason="bf16 CRF state, validated"):
            sing = ctx.enter_context(tc.tile_pool(name="sing", bufs=1))
            fpool = ctx.enter_context(tc.tile_pool(name="fpool", bufs=2))
            mpool = ctx.enter_context(tc.tile_pool(name="mpool", bufs=2))
            gpool = ctx.enter_context(tc.tile_pool(name="gpool", bufs=2))
            spool = ctx.enter_context(tc.tile_pool(name="spool", bufs=4))
            psum = ctx.enter_context(tc.tile_pool(name="psum", bufs=1,
                                                  space="PSUM"))

            # ---------- constants ----------
            transb = sing.tile([BL, K * K], f32)
            nc.sync.dma_start(transb[:], bass.AP(trans, 0, [[0, BL], [1, K * K]]))
            trv = transb[:].rearrange("p (j i) -> p j i", i=K)
            lenf_sb = sing.tile([BL, 1], f32)
            nc.sync.dma_start(lenf_sb[:], lenf[:, :])
            iota7 = sing.tile([BL, KA], f32)
            nc.sync.dma_start(iota7[:], bass.AP(c_iota7, 0, [[0, BL], [1, KA]]))
            hidx_sb = sing.tile([BL, 1], i32)
            nc.sync.dma_start(hidx_sb[:], hidx[:, :])
            oidx_sb = sing.tile([BL, 1], i32)
            nc.sync.dma_start(oidx_sb[:], oidx[:, :])
            lidx_sb = sing.tile([BL, 1], i32)
            nc.sync.dma_start(lidx_sb[:], lidx[:, :])
            isl = sing.tile([BL, 1], f32)
            nc.sync.dma_start(isl[:], islf[:, :])

            # E2[s, 0, j, i] = exp(trans[j, i]); E2[s, 1, a, b] = exp(trans[b, a])
            E2 = sing.tile([BL, 2, KA, KA], bf16)
            nc.scalar.activation(E2[:, 0, :, :], trv[:, 0:KA, 0:KA], AF.Exp)
            nc.scalar.activation(
                E2[:, 1, :, :],
                trv[:, 0:KA, 0:KA].rearrange("p j i -> p i j"), AF.Exp)
            E7E = sing.tile([BL, KA], bf16)      # exp(trans[j, START])
            nc.scalar.activation(E7E[:], trv[:, 0:KA, START:START + 1], AF.Exp)
            E8E = sing.tile([BL, KA], f32)       # exp(trans[STOP, j])
            nc.scalar.activation(E8E[:], trv[:, STOP:STOP + 1, 0:KA], AF.Exp)
            negG = sing.tile([BL, 1], f32)
            nc.vector.memset(negG[:], -G)

            # ---------- emissions: ef = exp(feat - G) ----------
            ef_a = sing.tile([BL, H + 1, KA], bf16)
            ef_b = sing.tile([BL, H, KA], bf16)
            for c in range(NCH):
                fa = fpool.tile([BL, CT, KA], bf16, tag="fa")
                nc.sync.dma_start(fa[:], fwd_f[:, c * CT:(c + 1) * CT, :])
                nc.scalar.activation(ef_a[:, c * CT:(c + 1) * CT, :], fa[:],
                                     AF.Exp, bias=negG[:, 0:1])
                fb = fpool.tile([BL, CT, KA], bf16, tag="fb")
                nc.sync.dma_start(fb[:], rev_f[:, c * CT:(c + 1) * CT, :])
                nc.scalar.activation(ef_b[:, c * CT:(c + 1) * CT, :], fb[:],
                                     AF.Exp, bias=negG[:, 0:1])
            nc.vector.memset(ef_a[:, H, :], 1.0)

            # ---------- dual chain ----------
            Dhist = sing.tile([BL, H + 1, 2, KA], bf16)
            MXb = sing.tile([BL, NEV, 2], f32)
            OFFb = sing.tile([BL, NEV + 1, 2], f32)

            nc.vector.tensor_tensor(
                Dhist[:, 0, 0, :], E7E[:], ef_a[:, 0, :], op=OP.mult)
            nc.vector.tensor_copy(Dhist[:, 0, 1, :], E8E[:])

            rcd = sing.tile([BL, 2], f32)
            for c in range(NCH):
                # Mab[s,l,0,j,i] = E[j,i]*ef_a[k,j]; Mab[s,l,1,a,b]=E[b,a]*ef_b[k-1,b]
                # with dual-step k = c*CT + l + 1
                Mab = mpool.tile([BL, CT, 2, KA, KA], bf16, tag="Mab")
                ka0 = c * CT + 1
                nc.gpsimd.tensor_tensor(
                    Mab[:, :, 0, :, :],
                    E2[:, 0, :, :].unsqueeze(1).broadcast_to([BL, CT, KA, KA]),
                    ef_a[:, ka0:ka0 + CT, :].unsqueeze(3)
                        .broadcast_to([BL, CT, KA, KA]),
                    op=OP.mult)
                nc.gpsimd.tensor_tensor(
                    Mab[:, :, 1, :, :],
                    E2[:, 1, :, :].unsqueeze(1).broadcast_to([BL, CT, KA, KA]),
                    ef_b[:, ka0 - 1:ka0 - 1 + CT, :].unsqueeze(2)
                        .broadcast_to([BL, CT, KA, KA]),
                    op=OP.mult)

                Xab = spool.tile([BL, 2, KA, KA], bf16, tag="Xab")
                for l in range(CT):
                    k = c * CT + l + 1
                    nc.vector.tensor_tensor(
                        Xab[:],
                        Mab[:, l, :, :, :],
                        Dhist[:, k - 1, :, :].unsqueeze(2)
                            .broadcast_to([BL, 2, KA, KA]),
                        op=OP.mult)
                    nc.vector.tensor_reduce(
                        out=Dhist[:, k, :, :], in_=Xab[:], axis=AX.X, op=OP.add)
                    if k % RN == 0:
                        ev = k // RN - 1
                        nc.vector.tensor_reduce(
                            out=MXb[:, ev, :], in_=Dhist[:, k, :, :],
                            axis=AX.X, op=OP.max)
                        nc.vector.reciprocal(rcd[:], MXb[:, ev, :])
                        nc.vector.tensor_tensor(
                            Dhist[:, k, :, :], Dhist[:, k, :, :],
                            rcd[:].unsqueeze(2).broadcast_to([BL, 2, KA]),
                            op=OP.mult)

            # ---------- offsets ----------
            LNb = sing.tile([BL, NEV, 2], f32)
            nc.scalar.activation(LNb[:], MXb[:], AF.Ln)
            onesb = sing.tile([BL, NEV], f32)
            nc.vector.memset(onesb[:], 1.0)
            nc.vector.memset(OFFb[:, 0, :], 0.0)
            for lane in range(2):
                nc.vector.tensor_tensor_scan(
                    out=OFFb[:, 1:NEV + 1, lane],
                    data0=onesb[:], data1=LNb[:, :, lane],
                    initial=0.0, op0=OP.mult, op1=OP.add)

            nc.sync.dma_start(
                bass.AP(dh_d, 0, [[(H + 1) * 2 * KA, BL],
                                  [1, (H + 1) * 2 * KA]]),
                Dhist[:].rearrange("p k l j -> p (k l j)"))
            nc.sync.dma_start(
                bass.AP(off_d, 0, [[(NEV + 1) * 2, BL], [1, (NEV + 1) * 2]]),
                OFFb[:].rearrange("p e l -> p (e l)"))

            # ---------- gold via PE ----------
            patT = sing.tile([TC_, NPS], f32)
            nc.sync.dma_start(patT[0:KA, :], patR[:, :])
            iota8T = sing.tile([TC_, KA + 1], f32)
            nc.sync.dma_start(
                iota8T[:], bass.AP(c_iota8p1, 0, [[0, TC_], [1, KA + 1]]))

            gps = psum.tile([128, BL * NPS], f32, tag="gps")
            gpv = gps[:].rearrange("p (s c) -> p s c", c=NPS)

            for c in range(NGC):
                ftc = gpool.tile([TC_, BL, KA], bf16, tag="ftc")
                nc.sync.dma_start(ftc[:], featT[:, c, :, :])
                ttc = gpool.tile([TC_, BL], bf16, tag="ttc")
                nc.sync.dma_start(ttc[:], tagT[:, c, :])
                tpc = gpool.tile([TC_, BL], bf16, tag="tpc")
                nc.sync.dma_start(tpc[:], tagpT[:, c, :])
                mkc = gpool.tile([TC_, BL], bf16, tag="mkc")
                nc.sync.dma_start(mkc[:], maskT[:, c, :])

                mtc = gpool.tile([TC_, BL], bf16, tag="mtc")
                nc.gpsimd.tensor_tensor(mtc[:], mkc[:], ttc[:], op=OP.mult)
                ohm = gpool.tile([TC_, BL, KA], bf16, tag="ohm")
                nc.vector.tensor_tensor(
                    ohm[:],
                    mtc[:].unsqueeze(2).broadcast_to([TC_, BL, KA]),
                    iota8T[:, 0:KA].unsqueeze(1).broadcast_to([TC_, BL, KA]),
                    op=OP.is_equal)
                ohp = gpool.tile([TC_, BL, KA + 1], bf16, tag="ohp")
                nc.vector.tensor_tensor(
                    ohp[:],
                    tpc[:].unsqueeze(2).broadcast_to([TC_, BL, KA + 1]),
                    iota8T[:].unsqueeze(1).broadcast_to([TC_, BL, KA + 1]),
                    op=OP.is_equal)

                for s in range(BL):
                    nc.tensor.matmul(
                        gpv[0:KA, s, 0:KA + 1],
                        lhsT=ohm[:, s, :], rhs=ohp[:, s, :],
                        start=(c == 0), stop=(c == NGC - 1),
                        skip_group_check=True)
                    nc.tensor.matmul(
                        gpv[0:KA, s, KA + 1:NPS],
                        lhsT=ohm[:, s, :], rhs=ftc[:, s, :],
                        start=(c == 0), stop=(c == NGC - 1),
                        skip_group_check=True)

            gj = spool.tile([128, BL, NPS], f32, tag="gj")
            nc.vector.tensor_tensor(
                gj[0:KA, :, :], gpv[0:KA, :, :],
                patT[0:KA, :].unsqueeze(1).broadcast_to([KA, BL, NPS]),
                op=OP.mult)
            gvecT = spool.tile([128, BL], f32, tag="gvecT")
            nc.vector.tensor_reduce(
                out=gvecT[0:KA, :], in_=gj[0:KA, :, :], axis=AX.X, op=OP.add)
            nc.sync.dma_start(
                bass.AP(gvec_d, 0, [[BL, KA], [1, BL]]), gvecT[0:KA, :])
            gvS = spool.tile([BL, KA], f32, tag="gvS")
            nc.sync.dma_start(gvS[:], bass.AP(gvec_d, 0, [[1, BL], [BL, KA]]))
            goldA = spool.tile([BL, 1], f32, tag="goldA")
            nc.vector.tensor_reduce(
                out=goldA[:], in_=gvS[:], axis=AX.X, op=OP.add)

            # ---------- last-tag term: trans[STOP, tag[len-1]] ----------
            tgl = spool.tile([BL, 1], bf16, tag="tgl")
            nc.gpsimd.indirect_dma_start(
                out=tgl[:], out_offset=None,
                in_=bass.AP(tagT, 0, [[1, TC_ * NGC * BL], [1, 1]]),
                in_offset=bass.IndirectOffsetOnAxis(ap=lidx_sb[:, 0:1], axis=0),
            )
            tglf = spool.tile([BL, 1], f32, tag="tglf")
            nc.vector.tensor_copy(tglf[:], tgl[:])
            ohl = spool.tile([BL, KA], f32, tag="ohl")
            nc.vector.scalar_tensor_tensor(
                out=ohl[:], in0=tglf[:].broadcast_to([BL, KA]), scalar=-1.0,
                in1=iota7[:], op0=OP.add, op1=OP.is_equal)
            jl = spool.tile([BL, KA], f32, tag="jl")
            nc.vector.tensor_tensor(
                jl[:], ohl[:], trv[:, STOP, 0:KA], op=OP.mult)
            lastp = spool.tile([BL, 1], f32, tag="lastp")
            nc.vector.tensor_reduce(out=lastp[:], in_=jl[:], axis=AX.X,
                                    op=OP.add)

            # ---------- extraction ----------
            Sg = spool.tile([BL, KA], bf16, tag="Sg")
            nc.gpsimd.indirect_dma_start(
                out=Sg[:], out_offset=None,
                in_=bass.AP(dh_d, 0, [[KA, BL * (H + 1) * 2], [1, KA]]),
                in_offset=bass.IndirectOffsetOnAxis(ap=hidx_sb[:, 0:1], axis=0),
            )
            offg = spool.tile([BL, 1], f32, tag="offg")
            nc.gpsimd.indirect_dma_start(
                out=offg[:], out_offset=None,
                in_=bass.AP(off_d, 0, [[1, BL * (NEV + 1) * 2], [1, 1]]),
                in_offset=bass.IndirectOffsetOnAxis(ap=oidx_sb[:, 0:1], axis=0),
            )
            # partner = isl ? alpha_m : E8
            amf = spool.tile([BL, KA], f32, tag="amf")
            nc.vector.tensor_copy(amf[:], Dhist[:, H - 1, 0, :])
            dmE = spool.tile([BL, KA], f32, tag="dmE")
            nc.vector.tensor_tensor(dmE[:], amf[:], E8E[:], op=OP.subtract)
            prt = spool.tile([BL, KA], f32, tag="prt")
            nc.vector.scalar_tensor_tensor(
                out=prt[:], in0=dmE[:], scalar=isl[:, 0:1], in1=E8E[:],
                op0=OP.mult, op1=OP.add)
            Sgf = spool.tile([BL, KA], f32, tag="Sgf")
            nc.vector.tensor_copy(Sgf[:], Sg[:])
            dotj = spool.tile([BL, KA], f32, tag="dotj")
            nc.vector.tensor_tensor(dotj[:], Sgf[:], prt[:], op=OP.mult)
            dots = spool.tile([BL, 1], f32, tag="dots")
            nc.vector.tensor_reduce(out=dots[:], in_=dotj[:], axis=AX.X,
                                    op=OP.add)
            lnv = spool.tile([BL, 1], f32, tag="lnv")
            nc.scalar.activation(lnv[:], dots[:], AF.Ln)
            # alpha_m (slot 1023) carries NEV-1 = 31 events -> OFF[31] lane 0
            at = spool.tile([BL, 1], f32, tag="at")
            nc.vector.scalar_tensor_tensor(
                out=at[:], in0=OFFb[:, NEV - 1, 0:1], scalar=isl[:, 0:1],
                in1=lnv[:], op0=OP.mult, op1=OP.add)
            f1 = spool.tile([BL, 1], f32, tag="f1")
            nc.vector.tensor_tensor(f1[:], at[:], offg[:], op=OP.add)
            fwdv = spool.tile([BL, 1], f32, tag="fwdv")
            nc.vector.scalar_tensor_tensor(
                out=fwdv[:], in0=lenf_sb[:], scalar=G, in1=f1[:],
                op0=OP.mult, op1=OP.add)

            # ---------- result ----------
            g2 = spool.tile([BL, 1], f32, tag="g2")
            nc.vector.tensor_tensor(g2[:], goldA[:], lastp[:], op=OP.add)
            res = spool.tile([BL, 1], f32, tag="res")
            nc.vector.tensor_tensor(res[:], fwdv[:], g2[:], op=OP.subtract)
            nc.sync.dma_start(outv[:, :], res[:])

    nc.finalize()
    return nc


def _prep_inputs(feats, transitions, tags, lengths):
    import ml_dtypes
    bf16 = ml_dtypes.bfloat16

    feats = np.asarray(feats, dtype=np.float32)
    transitions = np.asarray(transitions, dtype=np.float32)
    tags = np.asarray(tags).astype(np.int64)
    lengths = np.asarray(lengths).astype(np.int64)

    f7 = feats[:, :, :KA]
    fwd_f = np.ascontiguousarray(f7[:, :H]).astype(bf16)

    qi = np.arange(H)[None, :]
    src = lengths[:, None] - 1 - qi
    valid = src >= 0
    src_c = np.clip(src, 0, T - 1)
    gath = np.take_along_axis(f7, src_c[:, :, None], axis=1)
    rev_f = np.where(valid[:, :, None], gath, 0.0).astype(bf16)

    fT = np.ascontiguousarray(
        f7.reshape(B, NGC, TC_, KA).transpose(2, 1, 0, 3)).astype(bf16)
    tp1 = (tags + 1).astype(np.float32)
    tagT = np.ascontiguousarray(
        tp1.reshape(B, NGC, TC_).transpose(2, 1, 0)).astype(bf16)
    tprev = np.concatenate(
        [np.full((B, 1), START + 1, np.float32), tp1[:, :-1]], axis=1)
    tagpT = np.ascontiguousarray(
        tprev.reshape(B, NGC, TC_).transpose(2, 1, 0)).astype(bf16)
    mask = (np.arange(T)[None, :] < lengths[:, None]).astype(np.float32)
    maskT = np.ascontiguousarray(
        mask.reshape(B, NGC, TC_).transpose(2, 1, 0)).astype(bf16)

    pat = np.zeros((KA, NPS), dtype=np.float32)
    pat[:, :KA + 1] = transitions[:KA, :KA + 1]
    pat[:, KA + 1:] = np.eye(KA, dtype=np.float32)

    # per-core index vectors
    sarr = np.arange(BL, dtype=np.int64)
    lm1 = lengths - 1
    is_long = (lm1 > H - 1)
    kq = np.where(is_long, lengths - H, lm1)            # q or t
    hidx = (sarr[None, :] * 0 + 0)  # placeholder; built per core below
    prep = {
        "fwd_f": fwd_f, "rev_f": rev_f, "featT": fT, "tagT": tagT,
        "tagpT": tagpT, "maskT": maskT,
        "lenf": lengths.astype(np.float32).reshape(B, 1),
        "trans": transitions, "patR": pat,
        "lm1": lm1, "is_long": is_long, "kq": kq,
    }
    return prep


def kernel(feats, transitions, tags, lengths):
    prep = _prep_inputs(feats, transitions, tags, lengths)

    if "nc" not in _CACHE:
        _CACHE["nc"] = _build_bass()
    nc = _CACHE["nc"]

    from concourse.bass_utils import run_bass_kernel_spmd

    lm1, is_long, kq = prep["lm1"], prep["is_long"], prep["kq"]
    sarr = np.arange(BL, dtype=np.int64)

    in_maps = []
    for c in range(NCORES):
        sl = slice(c * BL, (c + 1) * BL)
        lm1c, islc, kqc = lm1[sl], is_long[sl], kq[sl]
        hidx = (sarr * (H + 1) * 2 + kqc * 2
                + np.where(islc, 1, 0)).astype(np.int32).reshape(BL, 1)
        oidx = (sarr * (NEV + 1) * 2 + (kqc // RN) * 2
                + np.where(islc, 1, 0)).astype(np.int32).reshape(BL, 1)
        lidx = ((lm1c % TC_) * (NGC * BL) + (lm1c // TC_) * BL
                + sarr).astype(np.int32).reshape(BL, 1)
        in_maps.append({
            "fwd_f": prep["fwd_f"][sl], "rev_f": prep["rev_f"][sl],
            "featT": np.ascontiguousarray(prep["featT"][:, :, sl]),
            "tagT": np.ascontiguousarray(prep["tagT"][:, :, sl]),
            "tagpT": np.ascontiguousarray(prep["tagpT"][:, :, sl]),
            "maskT": np.ascontiguousarray(prep["maskT"][:, :, sl]),
            "lenf": prep["lenf"][sl],
            "hidx": hidx, "oidx": oidx, "lidx": lidx,
            "islf": islc.astype(np.float32).reshape(BL, 1),
            "trans": prep["trans"], "patR": prep["patR"],
        })
    r = run_bass_kernel_spmd(nc, in_maps, core_ids=list(range(NCORES)),
                             trace=TRACE)
    if TRACE:
        _CACHE["last_result"] = r
    per_seq = np.concatenate([m["outv"].reshape(BL) for m in r.results])
    return np.float32(per_seq.mean(dtype=np.float64))
